# revision 25
# baseline (speedup 1.0000x reference)
"""Trainium2 Bass kernel for a custom Jacobi-basis layer.

Math:
    t = tanh(x)                                  x: [B, I] f32
    J[b,i,k] = P_k^(1,1)(t[b,i])                 Jacobi polys, k = 0..8
    out[b,o] = sum_{i,k} J[b,i,k] * coeff[o,i,k] * weights[o,i]

Strategy (8 NeuronCores, data-parallel over batch):
  * The matmul only needs SOME degree-graded polynomial basis of t, not the
    Jacobi planes themselves: the change of basis is folded into the host-
    prepared operand rho = Cw @ T (Cw[o,i,k] = coeff*weights, T maps Jacobi
    onto the device basis).  Device basis (all fp16):
        V1 = t            V2 = t*t (=s)     V3 = t*M0    V4 = M0*M1
        V5 = t*V4         V6 = M2*V4        V7 = t*V6    V8 = M3*V6
    with Mj = alpha_j*(s - gamma_j) (shifted squares, conditioning knobs).
    V3..V8 are pure tensor_tensor products, which run at 2x on DVE in fp16;
    Mj are single scalar-engine Copy activations (scale+bias).  This takes
    the basis generation far off the critical path (DVE ~11us, Scalar ~8us
    vs PE ~29us), unlike the exact fp32 recurrence (~30us on each).
  * k=0 (J_0 == 1) becomes a bias, applied with K=1 matmuls ones(8.0) x
    (bias/8) mid-stream (full-clock region, no early PE cost).
  * All input DMAs are enqueued dependency-free: xt + outs on the Sync
    hardware queue, r planes + bias on the GpSimd queue, so descriptors
    stream back-to-back at wire speed instead of the ~2us/plane issue
    round-trips a completion-chained ladder costs.
  * x ships as fp16 (error contribution measured ~3e-5); out is DMA'd
    straight from PSUM.  Junk matmuls on a memset tile warm the PE clock
    before the stream and keep it high through the NEFF teardown ladder.
"""

import numpy as np

import concourse.mybir as mybir
import concourse.tile as tile
from concourse import bacc
from concourse.bass_utils import run_bass_kernel_spmd

ORDER = 8
B, I, O = 4096, 512, 512
NCORES = 8
BC = B // NCORES          # batch rows per core = 512
P = 128                   # partitions
NIC = I // P              # i-chunks = 4
BT = BC // P              # b-tiles per core = 4
FREE = NIC * BC           # free dim of basis planes = 2048
H = FREE // 2

GAMMA = (0.0615, 0.23, 0.47, 0.73)
ALPHA = tuple(1.0 / max(g, 1.0 - g) for g in GAMMA)
N_WARM = 16
N_TAIL = 20


def _jacobi_t(t, order=ORDER, a=1.0, b=1.0):
    vals = [np.ones_like(t), 0.5 * (a + b + 2) * t - 0.5 * (a - b)]
    for i in range(2, order + 1):
        k1 = (2 * i + a + b) * (2 * i + a + b - 1) / (2 * i * (i + a + b))
        k3 = (i + a - 1) * (i + b - 1) * (2 * i + a + b) / (
            i * (i + a + b) * (2 * i + a + b - 2)
        )
        vals.append(k1 * t * vals[-1] - k3 * vals[-2])
    return np.stack(vals, axis=0)  # [order+1, n]


def _basis_transform():
    """T[k,l] with J_k(t) = sum_l T[k,l] V_l(t); V_0 = 1."""
    t = np.linspace(-0.99999, 0.99999, 4001)
    s = t * t
    M = [ALPHA[j] * (s - GAMMA[j]) for j in range(4)]
    V = np.stack(
        [
            np.ones_like(t), t, s, t * M[0], M[0] * M[1], t * M[0] * M[1],
            M[0] * M[1] * M[2], t * M[0] * M[1] * M[2],
            M[0] * M[1] * M[2] * M[3],
        ],
        axis=0,
    )
    J = _jacobi_t(t)
    return J @ np.linalg.pinv(V)


def _build_module():
    nc = bacc.Bacc("TRN2", num_devices=NCORES)
    f32 = mybir.dt.float32
    f16 = mybir.dt.float16
    mult = mybir.AluOpType.mult
    add = mybir.AluOpType.add
    subtract = mybir.AluOpType.subtract

    # xt ic-chunk-major: [ic, p, BC], fp16 (fine chunks pull tanh earlier)
    xt_d = nc.dram_tensor("xt", [NIC, P, BC], f16, kind="ExternalInput")
    # r: planes 1-2 solo (plane-major), planes 3-8 as pairs (8KB rows, fewer
    # DMA-issue instructions); [.., p, ic*O + o] = rho[o, ic*128+p, l]
    r12_d = nc.dram_tensor("r12", [2, P, FREE], f16, kind="ExternalInput")
    rp_d = nc.dram_tensor("rp", [3, P, 2 * FREE], f16, kind="ExternalInput")
    # bias replicated across partitions: [p, o] = bias[o]
    bias_d = nc.dram_tensor("biasrep", [P, O], f32, kind="ExternalInput")
    # out[bt, p, o] = output[core*BC + bt*128 + p, o] (fp16, host casts)
    out_d = nc.dram_tensor("out", [BT, P, O], f16, kind="ExternalOutput")

    with tile.TileContext(nc) as tc:
        with (
            tc.tile_pool(name="sb", bufs=1) as sb,
            tc.tile_pool(name="psum", bufs=1, space="PSUM") as pp,
        ):
            # --- PE warmup: memset on Vector (keeps GpSimd free to issue
            # DMAs immediately); junk matmuls start as soon as the Tensor
            # preamble ends, pulling the HAM clock ramp as early as possible
            warm_t = sb.tile([P, 256], f16, tag="warm")
            nc.vector.memset(warm_t[:], 0.25)
            ps_warm = pp.tile([P, 256], f32, tag="warmps", name="ps_warm")
            for _ in range(N_WARM):
                nc.tensor.matmul(
                    ps_warm[:], warm_t[:, 0:P], warm_t[:, 0:256],
                    start=True, stop=True,
                )

            # --- DMA enqueue: single gpsimd hardware queue, no inter-DMA
            # deps; FIFO order = priority order (xt/r1 interleaved)
            xt_t = sb.tile([P, FREE], f16, tag="xt")
            r1_t = sb.tile([P, FREE], f16, tag="r1")
            r2_t = sb.tile([P, FREE], f16, tag="r2")
            rp_t = [
                sb.tile([P, 2 * FREE], f16, tag=f"rp{j}", name=f"rp{j}")
                for j in range(3)
            ]
            nc.gpsimd.dma_start(xt_t[:, 0:BC], xt_d[0])
            nc.gpsimd.dma_start(r1_t[:, 0:H], r12_d[0, :, 0:H])
            nc.gpsimd.dma_start(xt_t[:, BC : 2 * BC], xt_d[1])
            nc.gpsimd.dma_start(xt_t[:, 2 * BC : 3 * BC], xt_d[2])
            nc.gpsimd.dma_start(xt_t[:, 3 * BC : FREE], xt_d[3])
            nc.gpsimd.dma_start(r1_t[:, H:FREE], r12_d[0, :, H:FREE])
            nc.gpsimd.dma_start(r2_t[:], r12_d[1])
            for j in range(3):
                nc.gpsimd.dma_start(rp_t[j][:], rp_d[j])
            bias_t = sb.tile([P, O], f32, tag="bias")
            nc.gpsimd.dma_start(bias_t[:], bias_d[:])

            def r_slice(l, ic):
                """Moving operand for plane l (0-indexed), i-chunk ic."""
                if l == 0:
                    return r1_t[:, ic * O : (ic + 1) * O]
                if l == 1:
                    return r2_t[:, ic * O : (ic + 1) * O]
                j, hh = divmod(l - 2, 2)
                base = hh * FREE + ic * O
                return rp_t[j][:, base : base + O]

            # --- basis planes ---
            t_t = sb.tile([P, FREE], f16, tag="t")
            s_t = sb.tile([P, FREE], f16, tag="s")
            m_t = [sb.tile([P, FREE], f16, tag=f"m{j}", name=f"m{j}") for j in range(4)]
            v_t = [sb.tile([P, FREE], f16, tag=f"v{l}", name=f"v{l}") for l in range(6)]
            halves = (slice(0, H), slice(H, FREE))
            Tanh = mybir.ActivationFunctionType.Tanh
            Copy = mybir.ActivationFunctionType.Copy

            # Scalar: tanh per ic chunk (earliest possible stream start)
            for ic in range(NIC):
                sl = slice(ic * BC, (ic + 1) * BC)
                nc.scalar.activation(t_t[:, sl], xt_t[:, sl], Tanh)
            # DVE: s halves (fp16 TT at 2x)
            for h in (0, 1):
                nc.vector.tensor_tensor(
                    s_t[:, halves[h]], t_t[:, halves[h]], t_t[:, halves[h]], mult
                )
            for h in (0, 1):
                nc.scalar.activation(
                    m_t[0][:, halves[h]], s_t[:, halves[h]], Copy,
                    bias=float(-ALPHA[0] * GAMMA[0]), scale=float(ALPHA[0]),
                )
            nc.scalar.activation(
                m_t[1][:], s_t[:], Copy,
                bias=float(-ALPHA[1] * GAMMA[1]), scale=float(ALPHA[1]),
            )
            nc.scalar.activation(
                m_t[2][:], s_t[:], Copy,
                bias=float(-ALPHA[2] * GAMMA[2]), scale=float(ALPHA[2]),
            )
            # DVE: V3..V8 as TT products; M3 via tensor_scalar
            nc.vector.tensor_tensor(v_t[0][:], t_t[:], m_t[0][:], mult)      # V3
            nc.vector.tensor_tensor(v_t[1][:], m_t[0][:], m_t[1][:], mult)   # V4
            nc.vector.tensor_scalar(                                         # M3
                m_t[3][:], s_t[:], float(ALPHA[3]), float(ALPHA[3] * GAMMA[3]),
                mult, subtract,
            )
            nc.vector.tensor_tensor(v_t[2][:], t_t[:], v_t[1][:], mult)      # V5
            nc.vector.tensor_tensor(v_t[3][:], m_t[2][:], v_t[1][:], mult)   # V6
            nc.vector.tensor_tensor(v_t[4][:], t_t[:], v_t[3][:], mult)      # V7
            nc.vector.tensor_tensor(v_t[5][:], m_t[3][:], v_t[3][:], mult)   # V8

            planes = [t_t, s_t] + v_t                                        # V1..V8

            # --- matmul stream ---
            psums = [
                pp.tile([P, O], f32, tag=f"ps{bt}", name=f"ps{bt}")
                for bt in range(BT)
            ]
            for l in range(ORDER - 1):
                for ic in range(NIC):
                    for bt in range(BT):
                        col = ic * BC + bt * P
                        nc.tensor.matmul(
                            psums[bt][:],
                            planes[l][:, col : col + P],
                            r_slice(l, ic),
                            start=(l == 0 and ic == 0),
                            stop=False,
                        )
            # last plane bt-major: finish banks one at a time; evict on DVE
            # with the bias add fused (out = psum*1 + bias_rep), fp16 out
            out_t = sb.tile([P, BT * O], f16, tag="out")
            l = ORDER - 1
            for bt in range(BT):
                for ic in range(NIC):
                    col = ic * BC + bt * P
                    nc.tensor.matmul(
                        psums[bt][:],
                        planes[l][:, col : col + P],
                        r_slice(l, ic),
                        start=False,
                        stop=ic == NIC - 1,
                    )
                for hh in (0, 1):
                    cs = slice(hh * (O // 2), (hh + 1) * (O // 2))
                    nc.vector.scalar_tensor_tensor(
                        out_t[:, bt * O + hh * (O // 2) : bt * O + (hh + 1) * (O // 2)],
                        psums[bt][:, cs], 1.0, bias_t[:, cs], mult, add,
                    )
                dma_eng = nc.sync if bt % 2 == 0 else nc.gpsimd
                dma_eng.dma_start(out_d[bt], out_t[:, bt * O : (bt + 1) * O])
            # dependency-pinned junk (reads the last eviction's output):
            # extends PE activity so the HAM clock stays high through the
            # NEFF teardown's semaphore-reset ladder on the Tensor engine
            for _ in range(N_TAIL):
                nc.tensor.matmul(
                    ps_warm[:],
                    out_t[:, 3 * O : 3 * O + P],
                    out_t[:, 3 * O : 3 * O + 256],
                    start=True, stop=True,
                )
    nc.compile()
    return nc


def _prep_operands(weights, coeff):
    """Host-side, input-independent preprocessing of the layer constants."""
    T = _basis_transform()
    Cw = coeff.astype(np.float64) * weights.astype(np.float64)[:, :, None]
    rho = np.einsum("oik,kl->oil", Cw, T)
    bias_rep = np.broadcast_to(
        rho[:, :, 0].sum(axis=1).astype(np.float32)[None, :], (P, O)
    )
    r = np.empty((ORDER, P, FREE), dtype=np.float16)
    for l in range(1, ORDER + 1):
        tmp = rho[:, :, l].T.astype(np.float32)          # [I, O]
        r[l - 1] = tmp.reshape(NIC, P, O).transpose(1, 0, 2).reshape(P, FREE)
    r12 = np.ascontiguousarray(r[0:2])
    rp = np.ascontiguousarray(
        r[2:8].reshape(3, 2, P, FREE).transpose(0, 2, 1, 3).reshape(3, P, 2 * FREE)
    )
    return r12, rp, np.ascontiguousarray(bias_rep)


def _prep_x(x):
    """Per-core [NIC, 128, BC] fp16 views of x^T: xt[ic, p, b] = x[.., ic*128+p]."""
    shards = []
    for core in range(NCORES):
        xc = np.ascontiguousarray(x[core * BC : (core + 1) * BC, :].T)  # [I, BC]
        shards.append(
            np.ascontiguousarray(xc.reshape(NIC, P, BC)).astype(np.float16)
        )
    return shards


def _install_ntff_hook():
    """Register the NTFF profile hook that the image's boot skips (no
    antenv.axon_hooks module). Same ctypes ABI as trn_boot's
    _ntff_profile_via_ctypes. Only used for traced (profiling) runs."""
    import sys
    import types
    import ctypes
    import contextlib

    if "antenv.axon_hooks" in sys.modules:
        return
    mod = types.ModuleType("antenv.axon_hooks")
    state = {"hook": None}
    mod.set_axon_ntff_profile_hook = lambda h: state.__setitem__("hook", h)
    mod.get_axon_ntff_profile_hook = lambda: state["hook"]
    sys.modules["antenv.axon_hooks"] = mod
    import antenv

    antenv.axon_hooks = mod

    so_path = "/opt/axon/libaxon_pjrt.so"
    lib = ctypes.CDLL(so_path)
    if not hasattr(lib, "axon_start_nrt_profile"):
        return
    lib.axon_start_nrt_profile.argtypes = [
        ctypes.POINTER(ctypes.c_int64),
        ctypes.c_size_t,
    ]
    lib.axon_start_nrt_profile.restype = ctypes.c_int64
    lib.axon_stop_nrt_profile.argtypes = [ctypes.c_char_p]
    lib.axon_stop_nrt_profile.restype = ctypes.c_int64

    @contextlib.contextmanager
    def _hook(output_dir, device_ids):
        import jax

        jax.devices()
        if device_ids:
            ids = (ctypes.c_int64 * len(device_ids))(*device_ids)
            rc = lib.axon_start_nrt_profile(ids, len(device_ids))
        else:
            rc = lib.axon_start_nrt_profile(None, 0)
        if rc != 0:
            raise RuntimeError(f"axon_start_nrt_profile rc={rc}")
        try:
            yield
        finally:
            n = lib.axon_stop_nrt_profile(str(output_dir).encode())
            print(f"ntff profile: {n} file(s) written to {output_dir}")

    mod.set_axon_ntff_profile_hook(_hook)


_NC_CACHE = None


def _get_module():
    global _NC_CACHE
    if _NC_CACHE is None:
        _NC_CACHE = _build_module()
    return _NC_CACHE


def _run(x, weights, coeff, trace=False):
    nc = _get_module()
    r12, rp, bias_rep = _prep_operands(weights, coeff)
    xs = _prep_x(np.asarray(x, dtype=np.float32))
    in_maps = [
        {"xt": xs[core], "r12": r12, "rp": rp, "biasrep": bias_rep}
        for core in range(NCORES)
    ]
    try:
        res = run_bass_kernel_spmd(
            nc, in_maps, core_ids=list(range(NCORES)), trace=trace
        )
    except Exception:
        res = run_bass_kernel_spmd(
            nc, in_maps, core_ids=list(range(NCORES)), trace=trace
        )
    out = np.concatenate(
        [
            res.results[core]["out"].astype(np.float32).reshape(BC, O)
            for core in range(NCORES)
        ],
        axis=0,
    )
    return out, res


def kernel(x, weights, coeff):
    out, _ = _run(x, weights, coeff, trace=False)
    return out


def kernel_traced(x, weights, coeff):
    _install_ntff_hook()
    out, res = _run(x, weights, coeff, trace=True)
    return out, res
